# revision 29
# baseline (speedup 1.0000x reference)
"""Trainium2 Bass kernel for an int4-quantized DeepseekMLP (gate/up/down + SiLU).

Strategy (8 NeuronCores, tensor-parallel over the intermediate dim):
  - Each core owns a slice of the 11008 intermediate rows (6x1408 + 2x1280,
    padded to a uniform 1408 with zero-scale rows so all cores run one NEFF).
  - x^T is prepared on the host (fp32 -> bf16 cast + transpose, same spirit
    as the host-side int4 unpack) so the device reads it with plain DMAs.
  - On device, per core:
      * int4 codes (host-unpacked to uint8) are dequantized on the DVE with two
        tensor_tensor ops per 128-row tile (subtract zero, multiply scale) using
        step-0 broadcast APs over the per-group scale/zero vectors.
      * Dequantized tiles are xbar-transposed SBUF->SBUF into strip tiles that
        token block 0 consumes DIRECTLY (the strip-pool write-after-read turns
        the transposes into just-in-time paced producers; the xbar pace of
        ~25us/pair stays ahead of the PE's 34us/pair demand). The strips are
        also stored to W^T DRAM in the background for token blocks 1-3.
      * The three matmuls run on the PE with everything in transposed layout
        (contraction dim on partitions). g^T/u^T accumulate in PSUM; SiLU runs
        on the scalar engine straight from PSUM; h^T = silu(g^T)*u^T on the
        DVE feeds the down matmul.
      * Partial down outputs (out^T) are ReduceScattered (bf16) over the 8
        cores along the output-feature dim, in chunks per token block so the
        collective drains while the PE still computes; the last block uses
        shrinking chunks so the final exposed collective is tiny.
  - Host reassembles the full [4, 1024, 4096] fp32 output from the 8 shards.

HWDGE ring discipline (FIFO per issuing engine, so emission order == service
order; only SP and ACT are HWDGE rings; the xbar transposes emit ~256B
packets and cap one ring at ~80GB/s, so they get the SP ring to themselves):
  - SP ring: all xbar transposes (gate/up pairs then down slabs), then the
    tb1-3 gate strip loads.
  - ACT ring: x^T first-half loads; W^T background stores (gate/up strips
    and down dT slabs); tb1-3 up strips; tb1-3 down strips; later x^T halves.
  - gpsimd SWDGE: codes/scale loads (nothing dependent interleaved, so they
    free-flow), x^T second halves, part stores, collectives, outT stores.
"""

import os

import numpy as np
import ml_dtypes

import concourse.bass as bass
import concourse.mybir as mybir
import concourse.tile as tile
from concourse.tile import add_dep_helper
from concourse import bacc
import concourse.bass_utils as bass_utils

N_CORES = 8
B, S = 4, 1024
T = B * S            # 4096 tokens
H = 4096             # hidden
INTER = 11008
ISL = 1408           # per-core inter slice (padded)
G = 64               # quant group size
TB = 1024            # token block
NC = 512             # matmul n-chunk (one PSUM bank)
NTB = T // TB        # 4
HT = H // 128        # 32 k-tiles for gate/up
IT = ISL // 128      # 11 i-tiles
NGH = H // G         # 64 groups along hidden (gate/up)
DG = ISL // G        # 22 groups along inter slice (down)
QH = 256             # down ho-slab height
NQ = H // QH         # 16 slabs

CORE_SIZES = [1408] * 6 + [1280] * 2
# RS chunking per token block (in ho-slabs of QH rows); the last block
# shrinks so the final exposed collective is tiny.
CHUNKS = {0: [8, 8], 1: [8, 8], 2: [8, 8], 3: [6, 5, 3, 2]}

dt = mybir.dt
Alu = mybir.AluOpType

LAST_RESULTS = None


def _build():
    nc = bacc.Bacc("TRN2", target_bir_lowering=False, debug=False,
                   num_devices=N_CORES)

    # x^T in partition-major chunk layout: [128, tb, n, k, t] flattened, so
    # each 512-token chunk load is contiguous per partition (big packets).
    xTd = nc.dram_tensor("xT", [128, NTB * 2 * HT * NC], dt.bfloat16,
                         kind="ExternalInput")
    gc = nc.dram_tensor("gc", [ISL, H], dt.uint8, kind="ExternalInput")
    uc = nc.dram_tensor("uc", [ISL, H], dt.uint8, kind="ExternalInput")
    dc = nc.dram_tensor("dc", [H, ISL], dt.uint8, kind="ExternalInput")
    gs = nc.dram_tensor("gs", [ISL, NGH], dt.float32, kind="ExternalInput")
    gz = nc.dram_tensor("gz", [ISL, NGH], dt.float32, kind="ExternalInput")
    us = nc.dram_tensor("us", [ISL, NGH], dt.float32, kind="ExternalInput")
    uz = nc.dram_tensor("uz", [ISL, NGH], dt.float32, kind="ExternalInput")
    dsc = nc.dram_tensor("dsc", [H, DG], dt.float32, kind="ExternalInput")
    dzr = nc.dram_tensor("dzr", [H, DG], dt.float32, kind="ExternalInput")
    outT = nc.dram_tensor("outT", [H // N_CORES, T], dt.float32,
                          kind="ExternalOutput")

    with tile.TileContext(nc) as tc:
        with (
            tc.tile_pool(name="dram", bufs=1, space="DRAM") as dram,
            tc.tile_pool(name="xn0", bufs=1) as xn0_pool,
            tc.tile_pool(name="xn1", bufs=1) as xn1_pool,
            tc.tile_pool(name="hp", bufs=2) as h_pool,
            tc.tile_pool(name="stage", bufs=2) as st_pool,   # gu dequant staging
            tc.tile_pool(name="dstage", bufs=4) as dst_pool,  # down staging
            tc.tile_pool(name="wstream", bufs=3) as w_pool,  # gu strips
            tc.tile_pool(name="dts", bufs=3) as dts_pool,    # down slab strips
            tc.tile_pool(name="dstream", bufs=2) as d_pool,  # tb1-3 down strips
            tc.tile_pool(name="codes", bufs=2) as c_pool,
            tc.tile_pool(name="sz", bufs=2) as sz_pool,
            tc.tile_pool(name="act", bufs=2) as a_pool,
            tc.tile_pool(name="ob", bufs=2) as o_pool,
            tc.tile_pool(name="psgu", bufs=1, space="PSUM") as ps_gu,
            tc.tile_pool(name="psd", bufs=2, space="PSUM") as ps_d,
        ):
            # ---- x^T loads, one tile per 512-token n-chunk. The first chunk
            # rides the ACT ring, the second the SWDGE (behind the first
            # codes) so both halves land ~in parallel at startup.
            CH = HT * NC

            def load_xn1(tb, xn1):
                off = (tb * 2 + 1) * CH
                with nc.named_scope(f"xT_{tb}"):
                    nc.scalar.dma_start(
                        xn1.rearrange("p k t -> p (k t)"),
                        xTd[:, off:off + CH])

            def make_xT(tb, defer_second=False):
                xn0 = xn0_pool.tile([128, HT, NC], dt.bfloat16, tag="xn0",
                                    name=f"xn0_{tb}")
                xn1 = xn1_pool.tile([128, HT, NC], dt.bfloat16, tag="xn1",
                                    name=f"xn1_{tb}")
                off = tb * 2 * CH
                with nc.named_scope(f"xT_{tb}"):
                    nc.scalar.dma_start(
                        xn0.rearrange("p k t -> p (k t)"),
                        xTd[:, off:off + CH])
                if not defer_second:
                    load_xn1(tb, xn1)
                return xn0, xn1

            # ---- dequant: one 128-row tile -> (codes - zero) * scale
            def dequant_rows(codes_dram, s_dram, z_dram, it, width, ngroups,
                             tag, pool=None, eng=None, pfx=""):
                pool = pool or st_pool
                eng = eng or nc.vector
                cs = c_pool.tile([128, width], dt.uint8, tag=pfx + "codes",
                                 name=f"cs_{tag}")
                nc.gpsimd.dma_start(cs[:], codes_dram[it * 128:(it + 1) * 128, :])
                ssb = sz_pool.tile([128, ngroups], dt.float32, tag=pfx + "ssb",
                                   name=f"ssb_{tag}")
                zsb = sz_pool.tile([128, ngroups], dt.float32, tag=pfx + "zsb",
                                   name=f"zsb_{tag}")
                nc.gpsimd.dma_start(ssb[:], s_dram[it * 128:(it + 1) * 128, :])
                nc.gpsimd.dma_start(zsb[:], z_dram[it * 128:(it + 1) * 128, :])
                tmp = pool.tile([128, width], dt.bfloat16, tag="stg",
                                name=f"tmp_{tag}")
                wb = pool.tile([128, width], dt.bfloat16, tag="stg",
                               name=f"wb_{tag}")
                eng.tensor_tensor(
                    tmp.rearrange("p (g k) -> p g k", k=G),
                    cs.rearrange("p (g k) -> p g k", k=G),
                    zsb[:, :, None].broadcast_to([128, ngroups, G]),
                    op=Alu.subtract,
                )
                eng.tensor_tensor(
                    wb.rearrange("p (g k) -> p g k", k=G),
                    tmp.rearrange("p (g k) -> p g k", k=G),
                    ssb[:, :, None].broadcast_to([128, ngroups, G]),
                    op=Alu.mult,
                )
                return wb

            # gate/up: dequant, xbar-transpose into a strip tile that tb0
            # reads directly; store to W^T DRAM in the background (ACT).
            gT_dram, uT_dram = [], []
            gu_strips = []

            xn0_0, xn1_0 = make_xT(0)

            gu_store_q = []

            def dequant_gu_tile(it, nm, codes_d, s_d, z_d, lst):
                wb = dequant_rows(codes_d, s_d, z_d, it, H, NGH, f"{nm}{it}")
                wTs = w_pool.tile([128, HT, 128], dt.bfloat16,
                                  tag="wstrip", name=f"wTs_{nm}{it}")
                # split the pair's transposes across both HWDGE rings: the
                # gate tile rides SP, the up tile ACT, halving pair latency
                xbar_eng = nc.sync if nm == "g" else nc.scalar
                xbar_eng.dma_start(wTs[:, :, :], wb[:], transpose=True)
                wT_d = dram.tile([128, HT * 128], dt.bfloat16,
                                 tag=f"{nm}T{it}", name=f"{nm}T{it}")
                # store deferred to SWDGE after all codes/down emission so
                # the SP ring carries xbars only (pair pace 25us < PE 34us)
                gu_store_q.append((wT_d, wTs))
                lst.append(wT_d)
                return wTs

            with nc.named_scope("dequant_gu"):
                g0 = dequant_gu_tile(0, "g", gc, gs, gz, gT_dram)
                u0 = dequant_gu_tile(0, "u", uc, us, uz, uT_dram)
                gu_strips.append((g0, u0))

            with nc.named_scope("dequant_gu"):
                for it in range(1, IT):
                    g_ = dequant_gu_tile(it, "g", gc, gs, gz, gT_dram)
                    u_ = dequant_gu_tile(it, "u", uc, us, uz, uT_dram)
                    gu_strips.append((g_, u_))

            # down: dequant two 128-row tiles per ho-slab, xbar-transpose each
            # straight from SBUF staging into the slab strip ([128, IT, QH]);
            # tb0 reads the strips directly, and they are stored to W^T DRAM
            # for tb1-3.
            dT_dram = []
            dts_strips = []
            with nc.named_scope("dequant_d"):
                for q in range(NQ):
                    dTs = dts_pool.tile([128, IT, QH], dt.bfloat16,
                                        tag="dts", name=f"dTs_{q}")
                    for r in range(QH // 128):
                        ot = q * (QH // 128) + r
                        wb = dequant_rows(dc, dsc, dzr, ot, ISL, DG,
                                          f"d{ot}", pool=dst_pool, pfx="d")
                        nc.scalar.dma_start(
                            dTs[:, :, r * 128:(r + 1) * 128], wb[:],
                            transpose=True)
                    dT_d = dram.tile([128, IT * QH], dt.bfloat16,
                                     tag=f"dT{q}", name=f"dT{q}")
                    nc.scalar.dma_start(
                        dT_d[:], dTs.rearrange("p a b -> p (a b)"))
                    dT_dram.append(dT_d)
                    dts_strips.append(dTs)

            with nc.named_scope("gu_stores"):
                for wT_d, wTs in gu_store_q:
                    nc.sync.dma_start(
                        wT_d[:], wTs.rearrange("p a b -> p (a b)"))

            def down_slab(tb, q, qq, wdT, h3, part, first_down_mm,
                          last_up_mm):
                for ho in range(QH // 128):
                    dps = ps_d.tile([128, TB], dt.float32, tag="dps",
                                    name=f"dps_{tb}_{q}_{ho}")
                    for n in range(TB // NC):
                        nsl = bass.ts(n, NC)
                        for it in range(IT):
                            mm = nc.tensor.matmul(
                                dps[:, nsl],
                                wdT[:, it, ho * 128:(ho + 1) * 128],
                                h3[:, it, nsl],
                                start=(it == 0),
                                stop=(it == IT - 1),
                            )
                            if first_down_mm[0] is None:
                                first_down_mm[0] = mm
                                # keep the down block after this tb's
                                # gate/up matmuls in the PE stream
                                add_dep_helper(
                                    mm.ins, last_up_mm.ins, sync=False,
                                    reason="down after gateup")
                    ob = o_pool.tile([128, TB], dt.bfloat16, tag="ob",
                                     name=f"ob_{tb}_{q}_{ho}")
                    nc.scalar.copy(ob[:], dps[:])
                    nc.gpsimd.dma_start(
                        part[(qq * (QH // 128) + ho) * 128:
                             (qq * (QH // 128) + ho + 1) * 128, :],
                        ob[:],
                    )

            # ---- main loop over token blocks
            xT_next = (xn0_0, xn1_0)
            for tb in range(NTB):
                xn0, xn1 = xT_next
                tsl = slice(tb * TB, (tb + 1) * TB)

                h3 = h_pool.tile([128, IT, TB], dt.bfloat16, tag="h3",
                                 name=f"h3_{tb}")
                with nc.named_scope(f"gateup_{tb}"):
                    for it in range(IT):
                        if tb == 0:
                            wgT, wuT = gu_strips[it]
                        else:
                            wgT = w_pool.tile([128, HT, 128], dt.bfloat16,
                                              tag="wstrip",
                                              name=f"wgT_{tb}_{it}")
                            wuT = w_pool.tile([128, HT, 128], dt.bfloat16,
                                              tag="wstrip",
                                              name=f"wuT_{tb}_{it}")
                            nc.sync.dma_start(
                                wgT.rearrange("p a b -> p (a b)"),
                                gT_dram[it][:])
                            nc.scalar.dma_start(
                                wuT.rearrange("p a b -> p (a b)"),
                                uT_dram[it][:])

                        gps = ps_gu.tile([128, TB], dt.float32, tag="gps",
                                         name=f"gps_{tb}_{it}")
                        ups = ps_gu.tile([128, TB], dt.float32, tag="ups",
                                         name=f"ups_{tb}_{it}")
                        for n, xn in enumerate((xn0, xn1)):
                            nsl = bass.ts(n, NC)
                            for ht in range(HT):
                                nc.tensor.matmul(
                                    gps[:, nsl],
                                    wgT[:, ht, :],
                                    xn[:, ht, :],
                                    start=(ht == 0), stop=(ht == HT - 1),
                                )
                            for ht in range(HT):
                                last_up_mm = nc.tensor.matmul(
                                    ups[:, nsl],
                                    wuT[:, ht, :],
                                    xn[:, ht, :],
                                    start=(ht == 0), stop=(ht == HT - 1),
                                )
                        sil = a_pool.tile([128, TB], dt.bfloat16, tag="sil",
                                          name=f"sil_{tb}_{it}")
                        nc.scalar.activation(sil[:], gps[:],
                                             mybir.ActivationFunctionType.Silu)
                        nc.vector.tensor_tensor(h3[:, it, :], sil[:], ups[:],
                                                op=Alu.mult)

                chunk_sizes = CHUNKS[tb]
                first_down_mm = [None]
                q_base = 0
                with nc.named_scope(f"down_{tb}"):
                    for ck, csz in enumerate(chunk_sizes):
                        rows_per_chunk = csz * QH
                        part = dram.tile([rows_per_chunk, TB], dt.bfloat16,
                                         tag=f"part{tb}_{ck}",
                                         name=f"part{tb}_{ck}")
                        for qq in range(csz):
                            q = q_base + qq
                            if tb == 0:
                                wdT = dts_strips[q]
                            else:
                                wdT = d_pool.tile([128, IT, QH], dt.bfloat16,
                                                  tag="dstrip",
                                                  name=f"wdT_{tb}_{q}")
                                nc.scalar.dma_start(
                                    wdT.rearrange("p a b -> p (a b)"),
                                    dT_dram[q][:])
                            down_slab(tb, q, qq, wdT, h3, part,
                                      first_down_mm, last_up_mm)

                        rs_o = dram.tile([rows_per_chunk // N_CORES, TB],
                                         dt.bfloat16,
                                         tag=f"rs{tb}_{ck}",
                                         name=f"rs{tb}_{ck}")
                        nc.gpsimd.collective_compute(
                            "ReduceScatter",
                            Alu.add,
                            replica_groups=[list(range(N_CORES))],
                            ins=[part.opt()],
                            outs=[rs_o.opt()],
                        )
                        rpc = rows_per_chunk // N_CORES
                        oro = q_base * QH // N_CORES
                        nc.gpsimd.dma_start(
                            outT[oro:oro + rpc, tsl], rs_o[:])
                        q_base += csz

                # next tb's x^T loads go here, after this tb's down strips,
                # so their write-after-read wait never head-of-line-blocks
                # traffic that is needed earlier.
                if tb + 1 < NTB:
                    xT_next = make_xT(tb + 1)

    nc.compile()
    return nc


def _unpack_codes(Wq):
    """int32 [out, in/2] holding 0..255 byte values -> uint8 codes [out, in].
    Column 2j is the high nibble of byte j, column 2j+1 the low nibble."""
    b = Wq.astype(np.uint8)
    codes = np.empty((Wq.shape[0], Wq.shape[1] * 2), np.uint8)
    codes[:, 0::2] = (b >> 4) & 0xF
    codes[:, 1::2] = b & 0xF
    return codes


def _pad_rows(a, n):
    if a.shape[0] == n:
        return np.ascontiguousarray(a)
    pad = np.zeros((n - a.shape[0],) + a.shape[1:], a.dtype)
    return np.ascontiguousarray(np.concatenate([a, pad], axis=0))


def _pad_cols(a, n):
    if a.shape[1] == n:
        return np.ascontiguousarray(a)
    pad = np.zeros((a.shape[0], n - a.shape[1]), a.dtype)
    return np.ascontiguousarray(np.concatenate([a, pad], axis=1))


def kernel(x, gate_Wq, up_Wq, down_Wq, gate_scale, gate_zero,
           up_scale, up_zero, down_scale, down_zero):
    global LAST_RESULTS

    x2 = np.asarray(x, np.float32).reshape(T, H)
    # partition-major chunk layout: [128p, tb, n, k, t] so each 512-token
    # chunk is contiguous per partition
    xT_host = np.ascontiguousarray(
        x2.reshape(NTB, 2, NC, HT, 128).transpose(4, 0, 1, 3, 2)
    ).astype(ml_dtypes.bfloat16).reshape(128, NTB * 2 * HT * NC)
    g_codes = _unpack_codes(np.asarray(gate_Wq))
    u_codes = _unpack_codes(np.asarray(up_Wq))
    d_codes = _unpack_codes(np.asarray(down_Wq))

    starts = np.cumsum([0] + CORE_SIZES)
    in_maps = []
    for c in range(N_CORES):
        lo, hi = int(starts[c]), int(starts[c + 1])
        glo, ghi = lo // G, hi // G
        in_maps.append({
            "xT": xT_host,
            "gc": _pad_rows(g_codes[lo:hi], ISL),
            "uc": _pad_rows(u_codes[lo:hi], ISL),
            "dc": _pad_cols(d_codes[:, lo:hi], ISL),
            "gs": _pad_rows(np.asarray(gate_scale, np.float32)[lo:hi], ISL),
            "gz": _pad_rows(np.asarray(gate_zero, np.float32)[lo:hi], ISL),
            "us": _pad_rows(np.asarray(up_scale, np.float32)[lo:hi], ISL),
            "uz": _pad_rows(np.asarray(up_zero, np.float32)[lo:hi], ISL),
            "dsc": _pad_cols(np.asarray(down_scale, np.float32)[:, glo:ghi], DG),
            "dzr": _pad_cols(np.asarray(down_zero, np.float32)[:, glo:ghi], DG),
        })

    nc = _build()

    trace = os.environ.get("KERNEL_TRACE", "0") == "1"
    kw = {}
    if trace:
        kw = dict(trace=True, trace_cores=[0])
    res = bass_utils.run_bass_kernel_spmd(
        nc, in_maps, core_ids=list(range(N_CORES)), **kw)
    LAST_RESULTS = res

    # Reassemble: the per-tb RS chunking determines which global output
    # features each core's outT rows hold.
    out = np.empty((T, H), np.float32)
    for c in range(N_CORES):
        oc = res.results[c]["outT"]  # [512, T] fp32
        for tb in range(NTB):
            tsl = slice(tb * TB, (tb + 1) * TB)
            q_base = 0
            for csz in CHUNKS[tb]:
                rows_per_chunk = csz * QH
                rpc = rows_per_chunk // N_CORES
                oro = q_base * QH // N_CORES        # row offset inside outT
                gl0 = q_base * QH + c * rpc         # global feature offset
                out[tsl, gl0:gl0 + rpc] = oc[oro:oro + rpc, tsl].T
                q_base += csz
    return out.reshape(B, S, H)
